# revision 20
# baseline (speedup 1.0000x reference)
"""Trainium2 Bass kernel for 3x3 valid Conv2D (NCHW, OIHW), batch-parallel on 8 cores.

x(32,64,130,130) conv w(128,64,3,3) -> (32,128,128,128), plus bias(128,)
broadcast against the LAST axis (Wo) of the output (faithful to the
reference's torch-style broadcast, which requires Wo == K == 128).

Strategy per core (4 images):
  - x stored in SBUF with row-parity interleave: partitions 0-63 = channels
    (even image rows), partitions 64-127 = channels (odd image rows). A tap
    pair (u, u+1) then reads both halves at ONE free-dim offset, so two
    64-deep taps fuse into one 128-deep matmul -- no data duplication.
  - Per 16 output rows (one "hblk"): two PSUM supertiles [128, 1024] (2
    banks each: even-rows bank | odd-rows bank). 18 matmul slots per hblk
    (12 full pair-matmuls + 12 half-width singles that run pairwise
    concurrent in distinct PE row groups) = 100% PE array utilization.
  - Pairs-block and singles-block order alternates per hblk so the PE pays
    only ONE LDWEIGHTS width-transition bubble per hblk instead of two.
  - PSUM eviction is split across engines: Vector evicts supertile 0,
    Scalar (ACT) evicts supertile 1, each as a single [128,1024] fp32->bf16
    copy with a parity-interleaving destination AP. Output staged + stored
    as bf16 (halves HBM store traffic); host casts back to fp32.
  - Head: first x rows and the weight halves land via four parallel DMA
    queues (sync/gpsimd/scalar/tensor) so compute starts ~9.5us; warmup
    matmuls on a zero tile keep the PE HAM clock-gate warming during the
    DMA head.
  - Tail: the last hblk is emitted sub-major with per-supertile eviction +
    split stores on two queues, so the kernel tail is ~2us instead of ~8.
"""
import numpy as np

B, C, K, H, W = 32, 64, 128, 130, 130
HO = WO = 128
NCORES = 8
BLOC = B // NCORES  # 4 images per core
T = 65              # parity row-pairs (rows 0..129 -> 65 even + 65 odd)
NG = 16             # groups of 8 output rows per image
TC = 9              # row-pairs per x chunk (2 groups + 1 overlap row)
NCHUNK = 8
NWARM = 8           # warmup matmuls (HAM clock-gate)
COMPUTE = "bf16"

_CACHE = {}


def _build(with_bias: bool, compute: str = "bf16"):
    import concourse.tile as tile
    from concourse import bacc, mybir

    nc = bacc.Bacc("TRN2", target_bir_lowering=False, debug=False)
    f32 = mybir.dt.float32
    bf16 = mybir.dt.bfloat16
    cdt = mybir.dt.float32r if compute == "f32r" else mybir.dt.bfloat16

    x_d = nc.dram_tensor("xloc", [BLOC, 128, T * W], cdt, kind="ExternalInput")
    w_d = nc.dram_tensor("wpk", [128, 1152], cdt, kind="ExternalInput")
    o_d = nc.dram_tensor("out", [BLOC, K, HO, WO], bf16, kind="ExternalOutput")
    if with_bias:
        b_d = nc.dram_tensor("btile", [128, 512], f32, kind="ExternalInput")

    o_flat = o_d.ap().rearrange("b k i j -> b k (i j)")
    x_flat = x_d.ap().rearrange("b p (t j) -> b p t j", j=W)
    w_ap = w_d.ap()

    with tile.TileContext(nc) as tc:
        with (
            tc.tile_pool(name="wpool", bufs=1) as wpool,
            tc.tile_pool(name="xpool", bufs=12) as xpool,
            tc.tile_pool(name="xpool0", bufs=1) as xpool0,
            tc.tile_pool(name="spool", bufs=4) as spool,
            tc.tile_pool(name="psum", bufs=2, space="PSUM") as psum,
        ):
            # head: everything the first matmuls need is split into 64-
            # partition pieces spread over the three DMA rings (sync,
            # gpsimd, scalar) so each piece lands ~1.1us after ring start.
            # hblk0 runs singles-first, so the small singles-weight piece
            # (cols 768:1152) is needed first, pairs-weights ~1.3us later.
            wt = wpool.tile([128, 1152], cdt)
            if with_bias:
                bt = wpool.tile([128, 512], f32, tag="bias")
                nc.scalar.dma_start(bt[:], b_d.ap()[:, :])
            xa = xpool0.tile([128, 6 * W], cdt, tag="xa")
            xb = xpool0.tile([128, 5 * W], cdt, tag="xb")
            # sync ring
            nc.sync.dma_start(xa[0:64, :], x_flat[0, 0:64, 0:6, :])
            nc.sync.dma_start(wt[64:128, 0:768], w_ap[64:128, 0:768])
            nc.sync.dma_start(xb[0:64, :], x_flat[0, 0:64, 4:9, :])
            # gpsimd ring
            nc.gpsimd.dma_start(xa[64:128, :], x_flat[0, 64:128, 0:6, :])
            nc.gpsimd.dma_start(wt[64:128, 768:1152], w_ap[64:128, 768:1152])
            nc.gpsimd.dma_start(xb[64:128, :], x_flat[0, 64:128, 4:9, :])
            # scalar ring (then stores)
            nc.scalar.dma_start(wt[0:64, 768:1152], w_ap[0:64, 768:1152])
            nc.scalar.dma_start(wt[0:64, 0:768], w_ap[0:64, 0:768])

            # warm the PE clock gate (HAM) during the DMA head on a
            # zero-filled tile (memset on the otherwise-idle Vector engine)
            warmsrc = wpool.tile([128, 512], cdt, tag="warmsrc")
            nc.vector.memset(warmsrc[:], 0.0)
            wrm = psum.tile([128, 1024], f32, tag="e0")
            for i in range(NWARM):
                dst = wrm[:, 0:512] if i % 2 == 0 else wrm[:, 512:1024]
                nc.tensor.matmul(dst, warmsrc[:, 0:128], warmsrc[:],
                                 start=True, stop=True)

            flip = True  # True: singles-block first; False: pairs first

            for b in range(BLOC):
                xvs = []
                for c in range(NCHUNK):
                    if b == 0 and c == 0:
                        xvs.append(None)
                        continue
                    xt = xpool.tile([128, TC * W], cdt)
                    if b == 0 and c == 1:
                        # split across both rings so hblk1 isn't input-gated
                        nc.sync.dma_start(xt[0:64, :], x_flat[b, 0:64, 8:8 + TC, :])
                        nc.gpsimd.dma_start(xt[64:128, :], x_flat[b, 64:128, 8:8 + TC, :])
                    else:
                        nc.sync.dma_start(xt[:], x_flat[b, :, 8 * c:8 * c + TC, :])
                    xvs.append(xt[:].rearrange("p (t j) -> p t j", j=W))

                for hblk in range(NG // 2):
                    first = (b == 0 and hblk == 0)
                    last = (b == BLOC - 1 and hblk == NG // 2 - 1)
                    st = spool.tile([128, 2048], f32 if with_bias else bf16)
                    xv = xvs[hblk]
                    if xv is None:
                        xv0 = xa[:].rearrange("p (t j) -> p t j", j=W)
                        xv1 = xb[:].rearrange("p (t j) -> p t j", j=W)
                        sub_srcs = [(xv0, 0), (xv1, 0)]
                    else:
                        sub_srcs = [(xv, 0), (xv, 4)]
                    pt0 = psum.tile([128, 1024], f32, tag="e0")
                    pt1 = psum.tile([128, 1024], f32, tag="e1")
                    pts = [pt0, pt1]
                    subs = [
                        (pt0[:, 0:512], pt0[:, 512:1024]) + sub_srcs[0],
                        (pt1[:, 0:512], pt1[:, 512:1024]) + sub_srcs[1],
                    ]

                    def emit_pairs(which, first_mm, final_mm):
                        for vi in range(3):
                            for pe, po, xsv, lm in which:
                                nc.tensor.matmul(
                                    pe, wt[:, 128 * vi:128 * (vi + 1)],
                                    xsv[:, lm:lm + 4, vi:vi + 128],
                                    start=(first_mm and vi == 0),
                                    stop=(final_mm and vi == 2),
                                )
                                nc.tensor.matmul(
                                    po, wt[:, 384 + 128 * vi:384 + 128 * (vi + 1)],
                                    xsv[:, lm + 1:lm + 5, vi:vi + 128],
                                    start=(first_mm and vi == 0),
                                    stop=(final_mm and vi == 2),
                                )

                    def single_h0(sub, vi, first_mm, final_mm):
                        pe, po, xsv, lm = sub
                        nc.tensor.matmul(
                            pe, wt[0:64, 768 + 128 * vi:768 + 128 * (vi + 1)],
                            xsv[0:64, lm + 1:lm + 5, vi:vi + 128],
                            start=(first_mm and vi == 0),
                            stop=(final_mm and vi == 2),
                        )

                    def single_h64(sub, vi, first_mm, final_mm):
                        pe, po, xsv, lm = sub
                        nc.tensor.matmul(
                            po, wt[64:128, 768 + 128 * vi:768 + 128 * (vi + 1)],
                            xsv[64:128, lm:lm + 4, vi:vi + 128],
                            start=(first_mm and vi == 0),
                            stop=(final_mm and vi == 2),
                        )

                    def emit_singles(which, first_mm, final_mm):
                        for vi in range(3):
                            for sub in which:
                                single_h0(sub, vi, first_mm, final_mm)
                                single_h64(sub, vi, first_mm, final_mm)

                    sv = st[:].rearrange("p (s r par j) -> p s par r j",
                                         s=2, r=4, par=2, j=WO)
                    svr = st[:].rearrange("p (i j) -> p i j", j=WO)

                    def evict(s):
                        pt = pts[s]
                        if with_bias:
                            nc.vector.tensor_add(
                                svr[:, 8 * s:8 * s + 8:2, :], pt[:, 0:512], bt[:])
                            nc.vector.tensor_add(
                                svr[:, 8 * s + 1:8 * s + 8:2, :], pt[:, 512:1024], bt[:])
                        elif last:
                            # per-bank eviction on both engines in parallel
                            # so the final store launches ~0.7us after the
                            # last matmul instead of ~1.3us
                            pv = pt[:].rearrange("p (par r j) -> p par r j",
                                                 par=2, j=WO)
                            nc.vector.tensor_copy(svr[:, 8 * s:8 * s + 8:2, :],
                                                  pv[:, 0])
                            nc.scalar.copy(svr[:, 8 * s + 1:8 * s + 8:2, :],
                                           pv[:, 1])
                        else:
                            pv = pt[:].rearrange("p (par r j) -> p par r j",
                                                 par=2, j=WO)
                            if s == 0:
                                nc.vector.tensor_copy(sv[:, s], pv)
                            else:
                                nc.scalar.copy(sv[:, s], pv)

                    if first:
                        # hblk0: ladder the singles in DMA-piece arrival
                        # order -- h0/sub0 solo, then h64/sub0 concurrent
                        # with h0/sub1 (distinct row groups), then h64/sub1
                        # solo -- and only then the pairs, giving the pairs-
                        # weights pieces a ~2us arrival cushion.
                        for vi in range(3):
                            single_h0(subs[0], vi, True, False)
                        for vi in range(3):
                            single_h64(subs[0], vi, True, False)
                            single_h0(subs[1], vi, True, False)
                        for vi in range(3):
                            single_h64(subs[1], vi, True, False)
                        emit_pairs([subs[0]], False, True)
                        emit_pairs([subs[1]], False, True)
                    elif last:
                        # sub-major with pairs last: supertile 0 finishes,
                        # evicts and stores while supertile 1 computes, so
                        # the kernel tail is one sub's evict+store.
                        for s in (0, 1):
                            emit_singles([subs[s]], True, False)
                            emit_pairs([subs[s]], False, True)
                            if last:
                                evict(s)
                                base = 2048 * hblk + 1024 * s
                                if s == 0:
                                    nc.sync.dma_start(
                                        o_flat[b, :, base:base + 1024],
                                        st[:, 1024 * s:1024 * s + 1024])
                                else:
                                    nc.scalar.dma_start(
                                        o_flat[b, :, base:base + 512],
                                        st[:, 1024 * s:1024 * s + 512])
                                    nc.sync.dma_start(
                                        o_flat[b, :, base + 512:base + 1024],
                                        st[:, 1024 * s + 512:1024 * s + 1024])
                    else:
                        blocks = (emit_singles, emit_pairs) if flip \
                            else (emit_pairs, emit_singles)
                        blocks[0](subs, True, False)
                        blocks[1](subs, False, True)
                    flip = not flip

                    if not last:
                        evict(0)
                        evict(1)
                        nc.scalar.dma_start(
                            o_flat[b, :, 2048 * hblk:2048 * (hblk + 1)], st[:])
    nc.compile()
    return nc


def _get_nc(with_bias: bool, compute: str = None):
    compute = compute or COMPUTE
    key = ("conv", with_bias, compute)
    if key not in _CACHE:
        _CACHE[key] = _build(with_bias, compute)
    return _CACHE[key]


def _prep_inputs(x, weight, bias, with_bias, compute: str = None):
    compute = compute or COMPUTE
    xs = x.reshape(NCORES, BLOC, C, H, W)
    xr = np.empty((NCORES, BLOC, 128, T * W), np.float32)
    xr[:, :, 0:64] = xs[:, :, :, 0::2, :].reshape(NCORES, BLOC, C, T * W)
    xr[:, :, 64:128] = xs[:, :, :, 1::2, :].reshape(NCORES, BLOC, C, T * W)

    wkc = np.ascontiguousarray(weight.transpose(2, 3, 1, 0))  # [u, v, c, k]
    wpk = np.empty((128, 1152), np.float32)
    for v in range(3):
        wpk[0:64, 128 * v:128 * (v + 1)] = wkc[0, v]        # even pair lower: u0
        wpk[64:128, 128 * v:128 * (v + 1)] = wkc[1, v]      # even pair upper: u1
        wpk[0:64, 384 + 128 * v:384 + 128 * (v + 1)] = wkc[1, v]    # odd pair lower: u1
        wpk[64:128, 384 + 128 * v:384 + 128 * (v + 1)] = wkc[2, v]  # odd pair upper: u2
        wpk[0:64, 768 + 128 * v:768 + 128 * (v + 1)] = wkc[2, v]    # even single: u2
        wpk[64:128, 768 + 128 * v:768 + 128 * (v + 1)] = wkc[0, v]  # odd single: u0

    if compute == "bf16":
        import ml_dtypes
        xr = xr.astype(ml_dtypes.bfloat16)
        wpk = wpk.astype(ml_dtypes.bfloat16)
    in_maps = []
    for core in range(NCORES):
        m = {"xloc": xr[core], "wpk": wpk}
        if with_bias:
            m["btile"] = np.tile(bias, (128, 4))  # bias[j] along free dim
        in_maps.append(m)
    return in_maps


def kernel(x, weight, bias):
    from concourse.bass_utils import run_bass_kernel_spmd

    x = np.ascontiguousarray(np.asarray(x, dtype=np.float32))
    weight = np.asarray(weight, dtype=np.float32)
    bias = np.asarray(bias, dtype=np.float32)
    with_bias = bool(np.any(bias))

    nc = _get_nc(with_bias)
    in_maps = _prep_inputs(x, weight, bias, with_bias)
    res = run_bass_kernel_spmd(nc, in_maps, core_ids=list(range(NCORES)))
    out = np.empty((B, K, HO, WO), np.float32)
    for core in range(NCORES):
        out[core * BLOC:(core + 1) * BLOC] = np.asarray(
            res.results[core]["out"], dtype=np.float32)
    return out


# revision 23
# speedup vs baseline: 1.1984x; 1.1984x over previous
"""Trainium2 Bass kernel for 3x3 valid Conv2D (NCHW, OIHW), batch-parallel on 8 cores.

x(32,64,130,130) conv w(128,64,3,3) -> (32,128,128,128), plus bias(128,)
broadcast against the LAST axis (Wo) of the output (faithful to the
reference's torch-style broadcast, which requires Wo == K == 128).

Strategy per core (4 images):
  - x stored in SBUF with row-parity interleave: partitions 0-63 = channels
    (even image rows), partitions 64-127 = channels (odd image rows). A tap
    pair (u, u+1) then reads both halves at ONE free-dim offset, so two
    64-deep taps fuse into one 128-deep matmul -- no data duplication.
  - Per 16 output rows (one "hblk"): two PSUM supertiles [128, 1024] (2
    banks each: even-rows bank | odd-rows bank). 18 matmul slots per hblk
    (12 full pair-matmuls + 12 half-width singles that run pairwise
    concurrent in distinct PE row groups) = 100% PE array utilization.
  - Pairs-block and singles-block order alternates per hblk so the PE pays
    only ONE LDWEIGHTS width-transition bubble per hblk instead of two.
  - PSUM eviction is split across engines: Vector evicts supertile 0,
    Scalar (ACT) evicts supertile 1, each as a single [128,1024] fp32->bf16
    copy with a parity-interleaving destination AP. Output staged + stored
    as bf16 (halves HBM store traffic); host casts back to fp32.
  - Head: first x rows and the weight halves land via four parallel DMA
    queues (sync/gpsimd/scalar/tensor) so compute starts ~9.5us; warmup
    matmuls on a zero tile keep the PE HAM clock-gate warming during the
    DMA head.
  - Tail: the last hblk is emitted sub-major with per-supertile eviction +
    split stores on two queues, so the kernel tail is ~2us instead of ~8.
"""
import numpy as np

B, C, K, H, W = 32, 64, 128, 130, 130
HO = WO = 128
NCORES = 8
BLOC = B // NCORES  # 4 images per core
T = 65              # parity row-pairs (rows 0..129 -> 65 even + 65 odd)
NG = 16             # groups of 8 output rows per image
TC = 9              # row-pairs per x chunk (2 groups + 1 overlap row)
NCHUNK = 8
NWARM = 8           # warmup matmuls (HAM clock-gate)
COMPUTE = "bf16"

_CACHE = {}


def _build(with_bias: bool, compute: str = "bf16"):
    import concourse.tile as tile
    from concourse import bacc, mybir

    nc = bacc.Bacc("TRN2", target_bir_lowering=False, debug=False)
    f32 = mybir.dt.float32
    bf16 = mybir.dt.bfloat16
    cdt = mybir.dt.float32r if compute == "f32r" else mybir.dt.bfloat16

    x_d = nc.dram_tensor("xloc", [BLOC, 128, T * W], cdt, kind="ExternalInput")
    w_d = nc.dram_tensor("wpk", [128, 1152], cdt, kind="ExternalInput")
    o_d = nc.dram_tensor("out", [BLOC, K, HO, WO], bf16, kind="ExternalOutput")
    if with_bias:
        b_d = nc.dram_tensor("btile", [128, 512], f32, kind="ExternalInput")

    o_flat = o_d.ap().rearrange("b k i j -> b k (i j)")
    x_flat = x_d.ap().rearrange("b p (t j) -> b p t j", j=W)
    w_ap = w_d.ap()

    with tile.TileContext(nc) as tc:
        with (
            tc.tile_pool(name="wpool", bufs=1) as wpool,
            tc.tile_pool(name="xpool", bufs=12) as xpool,
            tc.tile_pool(name="xpool0", bufs=1) as xpool0,
            tc.tile_pool(name="spool", bufs=4) as spool,
            tc.tile_pool(name="psum", bufs=2, space="PSUM") as psum,
        ):
            # head: everything the first matmuls need is split into 64-
            # partition pieces spread over the three DMA rings (sync,
            # gpsimd, scalar) so each piece lands ~1.1us after ring start.
            # hblk0 runs singles-first, so the small singles-weight piece
            # (cols 768:1152) is needed first, pairs-weights ~1.3us later.
            wt = wpool.tile([128, 1152], cdt)
            if with_bias:
                bt = wpool.tile([128, 512], f32, tag="bias")
                nc.scalar.dma_start(bt[:], b_d.ap()[:, :])
            xa = xpool0.tile([128, 6 * W], cdt, tag="xa")
            xb = xpool0.tile([128, 5 * W], cdt, tag="xb")
            # sync ring
            nc.sync.dma_start(xa[0:64, :], x_flat[0, 0:64, 0:6, :])
            nc.sync.dma_start(wt[64:128, 0:768], w_ap[64:128, 0:768])
            nc.sync.dma_start(xb[0:64, :], x_flat[0, 0:64, 4:9, :])
            # gpsimd ring
            nc.gpsimd.dma_start(xa[64:128, :], x_flat[0, 64:128, 0:6, :])
            nc.gpsimd.dma_start(wt[64:128, 768:1152], w_ap[64:128, 768:1152])
            nc.gpsimd.dma_start(xb[64:128, :], x_flat[0, 64:128, 4:9, :])
            # scalar ring (then stores)
            nc.scalar.dma_start(wt[0:64, 768:1152], w_ap[0:64, 768:1152])
            nc.scalar.dma_start(wt[0:64, 0:768], w_ap[0:64, 0:768])

            # warm the PE clock gate (HAM) during the DMA head on a
            # zero-filled tile (memset on the otherwise-idle Vector engine)
            warmsrc = wpool.tile([128, 512], cdt, tag="warmsrc")
            nc.vector.memset(warmsrc[:], 0.0)
            wrm = psum.tile([128, 1024], f32, tag="e0")
            for i in range(NWARM):
                dst = wrm[:, 0:512] if i % 2 == 0 else wrm[:, 512:1024]
                nc.tensor.matmul(dst, warmsrc[:, 0:128], warmsrc[:],
                                 start=True, stop=True)

            flip = True  # True: singles-block first; False: pairs first

            for b in range(BLOC):
                xvs = []
                for c in range(NCHUNK):
                    if b == 0 and c == 0:
                        xvs.append(None)
                        continue
                    xt = xpool.tile([128, TC * W], cdt)
                    if b == 0 and c == 1:
                        # split across both rings so hblk1 isn't input-gated
                        nc.sync.dma_start(xt[0:64, :], x_flat[b, 0:64, 8:8 + TC, :])
                        nc.gpsimd.dma_start(xt[64:128, :], x_flat[b, 64:128, 8:8 + TC, :])
                    else:
                        nc.sync.dma_start(xt[:], x_flat[b, :, 8 * c:8 * c + TC, :])
                    xvs.append(xt[:].rearrange("p (t j) -> p t j", j=W))

                for hblk in range(NG // 2):
                    first = (b == 0 and hblk == 0)
                    last = (b == BLOC - 1 and hblk == NG // 2 - 1)
                    st = spool.tile([128, 2048], f32 if with_bias else bf16)
                    xv = xvs[hblk]
                    if xv is None:
                        xv0 = xa[:].rearrange("p (t j) -> p t j", j=W)
                        xv1 = xb[:].rearrange("p (t j) -> p t j", j=W)
                        sub_srcs = [(xv0, 0), (xv1, 0)]
                    else:
                        sub_srcs = [(xv, 0), (xv, 4)]
                    pt0 = psum.tile([128, 1024], f32, tag="e0")
                    pt1 = psum.tile([128, 1024], f32, tag="e1")
                    pts = [pt0, pt1]
                    subs = [
                        (pt0[:, 0:512], pt0[:, 512:1024]) + sub_srcs[0],
                        (pt1[:, 0:512], pt1[:, 512:1024]) + sub_srcs[1],
                    ]

                    def emit_pairs(which, first_mm, final_mm):
                        for vi in range(3):
                            for pe, po, xsv, lm in which:
                                nc.tensor.matmul(
                                    pe, wt[:, 128 * vi:128 * (vi + 1)],
                                    xsv[:, lm:lm + 4, vi:vi + 128],
                                    start=(first_mm and vi == 0),
                                    stop=(final_mm and vi == 2),
                                )
                                nc.tensor.matmul(
                                    po, wt[:, 384 + 128 * vi:384 + 128 * (vi + 1)],
                                    xsv[:, lm + 1:lm + 5, vi:vi + 128],
                                    start=(first_mm and vi == 0),
                                    stop=(final_mm and vi == 2),
                                )

                    def single_h0(sub, vi, first_mm, final_mm):
                        pe, po, xsv, lm = sub
                        nc.tensor.matmul(
                            pe, wt[0:64, 768 + 128 * vi:768 + 128 * (vi + 1)],
                            xsv[0:64, lm + 1:lm + 5, vi:vi + 128],
                            start=(first_mm and vi == 0),
                            stop=(final_mm and vi == 2),
                        )

                    def single_h64(sub, vi, first_mm, final_mm):
                        pe, po, xsv, lm = sub
                        nc.tensor.matmul(
                            po, wt[64:128, 768 + 128 * vi:768 + 128 * (vi + 1)],
                            xsv[64:128, lm:lm + 4, vi:vi + 128],
                            start=(first_mm and vi == 0),
                            stop=(final_mm and vi == 2),
                        )

                    def emit_singles(which, first_mm, final_mm):
                        for vi in range(3):
                            for sub in which:
                                single_h0(sub, vi, first_mm, final_mm)
                                single_h64(sub, vi, first_mm, final_mm)

                    sv = st[:].rearrange("p (s r par j) -> p s par r j",
                                         s=2, r=4, par=2, j=WO)
                    svr = st[:].rearrange("p (i j) -> p i j", j=WO)

                    def evict(s):
                        pt = pts[s]
                        if with_bias:
                            nc.vector.tensor_add(
                                svr[:, 8 * s:8 * s + 8:2, :], pt[:, 0:512], bt[:])
                            nc.vector.tensor_add(
                                svr[:, 8 * s + 1:8 * s + 8:2, :], pt[:, 512:1024], bt[:])
                        elif last:
                            # per-bank eviction on both engines in parallel
                            # so the final store launches ~0.7us after the
                            # last matmul instead of ~1.3us
                            pv = pt[:].rearrange("p (par r j) -> p par r j",
                                                 par=2, j=WO)
                            nc.vector.tensor_copy(svr[:, 8 * s:8 * s + 8:2, :],
                                                  pv[:, 0])
                            nc.scalar.copy(svr[:, 8 * s + 1:8 * s + 8:2, :],
                                           pv[:, 1])
                        else:
                            pv = pt[:].rearrange("p (par r j) -> p par r j",
                                                 par=2, j=WO)
                            if s == 0:
                                nc.vector.tensor_copy(sv[:, s], pv)
                            else:
                                nc.scalar.copy(sv[:, s], pv)

                    if first:
                        # hblk0: ladder the singles in DMA-piece arrival
                        # order -- h0/sub0 solo, then h64/sub0 concurrent
                        # with h0/sub1 (distinct row groups), then h64/sub1
                        # solo -- and only then the pairs, giving the pairs-
                        # weights pieces a ~2us arrival cushion.
                        for vi in range(3):
                            single_h0(subs[0], vi, True, False)
                        for vi in range(3):
                            single_h64(subs[0], vi, True, False)
                            single_h0(subs[1], vi, True, False)
                        for vi in range(3):
                            single_h64(subs[1], vi, True, False)
                        emit_pairs([subs[0]], False, True)
                        emit_pairs([subs[1]], False, True)
                    elif last:
                        # sub-major with pairs last: supertile 0 finishes,
                        # evicts and stores while supertile 1 computes, so
                        # the kernel tail is one sub's evict+store.
                        for s in (0, 1):
                            emit_singles([subs[s]], True, False)
                            emit_pairs([subs[s]], False, True)
                            if last:
                                evict(s)
                                base = 2048 * hblk + 1024 * s
                                if s == 0:
                                    nc.sync.dma_start(
                                        o_flat[b, :, base:base + 1024],
                                        st[:, 1024 * s:1024 * s + 1024])
                                else:
                                    nc.scalar.dma_start(
                                        o_flat[b, :, base:base + 512],
                                        st[:, 1024 * s:1024 * s + 512])
                                    nc.sync.dma_start(
                                        o_flat[b, :, base + 512:base + 1024],
                                        st[:, 1024 * s + 512:1024 * s + 1024])
                    else:
                        blocks = (emit_singles, emit_pairs) if flip \
                            else (emit_pairs, emit_singles)
                        blocks[0](subs, True, False)
                        blocks[1](subs, False, True)
                    flip = not flip

                    if not last:
                        evict(0)
                        evict(1)
                        nc.scalar.dma_start(
                            o_flat[b, :, 2048 * hblk:2048 * (hblk + 1)], st[:])
    nc.compile()
    return nc


def _get_nc(with_bias: bool, compute: str = None):
    compute = compute or COMPUTE
    key = ("conv", with_bias, compute)
    if key not in _CACHE:
        _CACHE[key] = _build(with_bias, compute)
    return _CACHE[key]


def _prep_inputs(x, weight, bias, with_bias, compute: str = None):
    compute = compute or COMPUTE
    xs = x.reshape(NCORES, BLOC, C, H, W)
    xr = np.empty((NCORES, BLOC, 128, T * W), np.float32)
    xr[:, :, 0:64] = xs[:, :, :, 0::2, :].reshape(NCORES, BLOC, C, T * W)
    xr[:, :, 64:128] = xs[:, :, :, 1::2, :].reshape(NCORES, BLOC, C, T * W)

    wkc = np.ascontiguousarray(weight.transpose(2, 3, 1, 0))  # [u, v, c, k]
    wpk = np.empty((128, 1152), np.float32)
    for v in range(3):
        wpk[0:64, 128 * v:128 * (v + 1)] = wkc[0, v]        # even pair lower: u0
        wpk[64:128, 128 * v:128 * (v + 1)] = wkc[1, v]      # even pair upper: u1
        wpk[0:64, 384 + 128 * v:384 + 128 * (v + 1)] = wkc[1, v]    # odd pair lower: u1
        wpk[64:128, 384 + 128 * v:384 + 128 * (v + 1)] = wkc[2, v]  # odd pair upper: u2
        wpk[0:64, 768 + 128 * v:768 + 128 * (v + 1)] = wkc[2, v]    # even single: u2
        wpk[64:128, 768 + 128 * v:768 + 128 * (v + 1)] = wkc[0, v]  # odd single: u0

    if compute == "bf16":
        import ml_dtypes
        xr = xr.astype(ml_dtypes.bfloat16)
        wpk = wpk.astype(ml_dtypes.bfloat16)
    in_maps = []
    for core in range(NCORES):
        m = {"xloc": xr[core], "wpk": wpk}
        if with_bias:
            m["btile"] = np.tile(bias, (128, 4))  # bias[j] along free dim
        in_maps.append(m)
    return in_maps


def kernel(x, weight, bias):
    from concourse.bass_utils import run_bass_kernel_spmd

    x = np.ascontiguousarray(np.asarray(x, dtype=np.float32))
    weight = np.asarray(weight, dtype=np.float32)
    bias = np.asarray(bias, dtype=np.float32)
    with_bias = bool(np.any(bias))

    nc = _get_nc(with_bias)
    in_maps = _prep_inputs(x, weight, bias, with_bias)
    res = run_bass_kernel_spmd(nc, in_maps, core_ids=list(range(NCORES)))
    out = np.empty((B, K, HO, WO), np.float32)
    for core in range(NCORES):
        out[core * BLOC:(core + 1) * BLOC] = np.asarray(
            res.results[core]["out"], dtype=np.float32)
    return out


# revision 25
# speedup vs baseline: 1.2241x; 1.0214x over previous
"""Trainium2 Bass kernel for 3x3 valid Conv2D (NCHW, OIHW), batch-parallel on 8 cores.

x(32,64,130,130) conv w(128,64,3,3) -> (32,128,128,128), plus bias(128,)
broadcast against the LAST axis (Wo) of the output (faithful to the
reference's torch-style broadcast, which requires Wo == K == 128).

Strategy per core (4 images):
  - x stored in SBUF with row-parity interleave: partitions 0-63 = channels
    (even image rows), partitions 64-127 = channels (odd image rows). A tap
    pair (u, u+1) then reads both halves at ONE free-dim offset, so two
    64-deep taps fuse into one 128-deep matmul -- no data duplication.
  - Per 16 output rows (one "hblk"): two PSUM supertiles [128, 1024] (2
    banks each: even-rows bank | odd-rows bank). 18 matmul slots per hblk
    (12 full pair-matmuls + 12 half-width singles that run pairwise
    concurrent in distinct PE row groups) = 100% PE array utilization.
  - Pairs-block and singles-block order alternates per hblk so the PE pays
    only ONE LDWEIGHTS width-transition bubble per hblk instead of two.
  - PSUM eviction is split across engines: Vector evicts supertile 0,
    Scalar (ACT) evicts supertile 1, each as a single [128,1024] fp32->bf16
    copy with a parity-interleaving destination AP. Output staged + stored
    as bf16 (halves HBM store traffic); host casts back to fp32.
  - Head: first x rows and the weight halves land via four parallel DMA
    queues (sync/gpsimd/scalar/tensor) so compute starts ~9.5us; warmup
    matmuls on a zero tile keep the PE HAM clock-gate warming during the
    DMA head.
  - Tail: the last hblk is emitted sub-major with per-supertile eviction +
    split stores on two queues, so the kernel tail is ~2us instead of ~8.
"""
import numpy as np

B, C, K, H, W = 32, 64, 128, 130, 130
HO = WO = 128
NCORES = 8
BLOC = B // NCORES  # 4 images per core
T = 65              # parity row-pairs (rows 0..129 -> 65 even + 65 odd)
NG = 16             # groups of 8 output rows per image
TC = 9              # row-pairs per x chunk (2 groups + 1 overlap row)
NCHUNK = 8
NWARM = 8           # warmup matmuls (HAM clock-gate)
COMPUTE = "bf16"

_CACHE = {}


def _build(with_bias: bool, compute: str = "bf16"):
    import concourse.tile as tile
    from concourse import bacc, mybir

    nc = bacc.Bacc("TRN2", target_bir_lowering=False, debug=False)
    f32 = mybir.dt.float32
    bf16 = mybir.dt.bfloat16
    cdt = mybir.dt.float32r if compute == "f32r" else mybir.dt.bfloat16

    x_d = nc.dram_tensor("xloc", [BLOC, 128, T * W], cdt, kind="ExternalInput")
    w_d = nc.dram_tensor("wpk", [128, 1152], cdt, kind="ExternalInput")
    o_d = nc.dram_tensor("out", [BLOC, K, HO, WO], bf16, kind="ExternalOutput")
    if with_bias:
        b_d = nc.dram_tensor("btile", [128, 512], f32, kind="ExternalInput")

    o_flat = o_d.ap().rearrange("b k i j -> b k (i j)")
    x_flat = x_d.ap().rearrange("b p (t j) -> b p t j", j=W)
    w_ap = w_d.ap()

    with tile.TileContext(nc) as tc:
        with (
            tc.tile_pool(name="wpool", bufs=1) as wpool,
            tc.tile_pool(name="xpool", bufs=12) as xpool,
            tc.tile_pool(name="xpool0", bufs=1) as xpool0,
            tc.tile_pool(name="spool", bufs=4) as spool,
            tc.tile_pool(name="psum", bufs=2, space="PSUM") as psum,
        ):
            # head: everything the first matmuls need is split into 64-
            # partition pieces spread over the three DMA rings (sync,
            # gpsimd, scalar) so each piece lands ~1.1us after ring start.
            # hblk0 runs singles-first, so the small singles-weight piece
            # (cols 768:1152) is needed first, pairs-weights ~1.3us later.
            wt = wpool.tile([128, 1152], cdt)
            if with_bias:
                bt = wpool.tile([128, 512], f32, tag="bias")
                nc.scalar.dma_start(bt[:], b_d.ap()[:, :])
            xa = xpool0.tile([128, 6 * W], cdt, tag="xa")
            xb = xpool0.tile([128, 5 * W], cdt, tag="xb")
            # sync ring
            nc.sync.dma_start(xa[0:64, :], x_flat[0, 0:64, 0:6, :])
            nc.sync.dma_start(wt[64:128, 0:768], w_ap[64:128, 0:768])
            nc.sync.dma_start(xb[0:64, :], x_flat[0, 0:64, 4:9, :])
            # gpsimd ring
            nc.gpsimd.dma_start(xa[64:128, :], x_flat[0, 64:128, 0:6, :])
            nc.gpsimd.dma_start(wt[64:128, 768:1152], w_ap[64:128, 768:1152])
            nc.gpsimd.dma_start(xb[64:128, :], x_flat[0, 64:128, 4:9, :])
            # scalar ring (then stores)
            nc.scalar.dma_start(wt[0:64, 768:1152], w_ap[0:64, 768:1152])
            nc.scalar.dma_start(wt[0:64, 0:768], w_ap[0:64, 0:768])

            # warm the PE clock gate (HAM) during the DMA head on a
            # zero-filled tile (memset on the otherwise-idle Vector engine)
            warmsrc = wpool.tile([128, 512], cdt, tag="warmsrc")
            nc.vector.memset(warmsrc[:], 0.0)
            wrm = psum.tile([128, 1024], f32, tag="e0")
            for i in range(NWARM):
                dst = wrm[:, 0:512] if i % 2 == 0 else wrm[:, 512:1024]
                nc.tensor.matmul(dst, warmsrc[:, 0:128], warmsrc[:],
                                 start=True, stop=True)

            flip = True  # True: singles-block first; False: pairs first

            for b in range(BLOC):
                xvs = []
                for c in range(NCHUNK):
                    if b == 0 and c == 0:
                        xvs.append(None)
                        continue
                    xt = xpool.tile([128, TC * W], cdt)
                    if b == 0 and c == 1:
                        # split across both rings so hblk1 isn't input-gated
                        nc.sync.dma_start(xt[0:64, :], x_flat[b, 0:64, 8:8 + TC, :])
                        nc.gpsimd.dma_start(xt[64:128, :], x_flat[b, 64:128, 8:8 + TC, :])
                    else:
                        nc.sync.dma_start(xt[:], x_flat[b, :, 8 * c:8 * c + TC, :])
                    xvs.append(xt[:].rearrange("p (t j) -> p t j", j=W))

                for hblk in range(NG // 2):
                    first = (b == 0 and hblk == 0)
                    last = (b == BLOC - 1 and hblk == NG // 2 - 1)
                    st = spool.tile([128, 2048], f32 if with_bias else bf16)
                    xv = xvs[hblk]
                    if xv is None:
                        xv0 = xa[:].rearrange("p (t j) -> p t j", j=W)
                        xv1 = xb[:].rearrange("p (t j) -> p t j", j=W)
                        sub_srcs = [(xv0, 0), (xv1, 0)]
                    else:
                        sub_srcs = [(xv, 0), (xv, 4)]
                    pt0 = psum.tile([128, 1024], f32, tag="e0")
                    pt1 = psum.tile([128, 1024], f32, tag="e1")
                    pts = [pt0, pt1]
                    subs = [
                        (pt0[:, 0:512], pt0[:, 512:1024]) + sub_srcs[0],
                        (pt1[:, 0:512], pt1[:, 512:1024]) + sub_srcs[1],
                    ]

                    def emit_pairs(which, first_mm, final_mm):
                        for vi in range(3):
                            for pe, po, xsv, lm in which:
                                nc.tensor.matmul(
                                    pe, wt[:, 128 * vi:128 * (vi + 1)],
                                    xsv[:, lm:lm + 4, vi:vi + 128],
                                    start=(first_mm and vi == 0),
                                    stop=(final_mm and vi == 2),
                                )
                                nc.tensor.matmul(
                                    po, wt[:, 384 + 128 * vi:384 + 128 * (vi + 1)],
                                    xsv[:, lm + 1:lm + 5, vi:vi + 128],
                                    start=(first_mm and vi == 0),
                                    stop=(final_mm and vi == 2),
                                )

                    def single_h0(sub, vi, first_mm, final_mm):
                        pe, po, xsv, lm = sub
                        nc.tensor.matmul(
                            pe, wt[0:64, 768 + 128 * vi:768 + 128 * (vi + 1)],
                            xsv[0:64, lm + 1:lm + 5, vi:vi + 128],
                            start=(first_mm and vi == 0),
                            stop=(final_mm and vi == 2),
                        )

                    def single_h64(sub, vi, first_mm, final_mm):
                        pe, po, xsv, lm = sub
                        nc.tensor.matmul(
                            po, wt[64:128, 768 + 128 * vi:768 + 128 * (vi + 1)],
                            xsv[64:128, lm:lm + 4, vi:vi + 128],
                            start=(first_mm and vi == 0),
                            stop=(final_mm and vi == 2),
                        )

                    def emit_singles(which, first_mm, final_mm):
                        for vi in range(3):
                            for sub in which:
                                single_h0(sub, vi, first_mm, final_mm)
                                single_h64(sub, vi, first_mm, final_mm)

                    sv = st[:].rearrange("p (s r par j) -> p s par r j",
                                         s=2, r=4, par=2, j=WO)
                    svr = st[:].rearrange("p (i j) -> p i j", j=WO)

                    def evict(s):
                        pt = pts[s]
                        if with_bias:
                            nc.vector.tensor_add(
                                svr[:, 8 * s:8 * s + 8:2, :], pt[:, 0:512], bt[:])
                            nc.vector.tensor_add(
                                svr[:, 8 * s + 1:8 * s + 8:2, :], pt[:, 512:1024], bt[:])
                        elif last:
                            # per-bank eviction on both engines in parallel
                            # so the final store launches ~0.7us after the
                            # last matmul instead of ~1.3us
                            pv = pt[:].rearrange("p (par r j) -> p par r j",
                                                 par=2, j=WO)
                            nc.vector.tensor_copy(svr[:, 8 * s:8 * s + 8:2, :],
                                                  pv[:, 0])
                            nc.scalar.copy(svr[:, 8 * s + 1:8 * s + 8:2, :],
                                           pv[:, 1])
                        else:
                            pv = pt[:].rearrange("p (par r j) -> p par r j",
                                                 par=2, j=WO)
                            if s == 0:
                                nc.vector.tensor_copy(sv[:, s], pv)
                            else:
                                nc.scalar.copy(sv[:, s], pv)

                    if first:
                        # hblk0: ladder the singles in DMA-piece arrival
                        # order -- h0/sub0 solo, then h64/sub0 concurrent
                        # with h0/sub1 (distinct row groups), then h64/sub1
                        # solo -- and only then the pairs, giving the pairs-
                        # weights pieces a ~2us arrival cushion.
                        for vi in range(3):
                            single_h0(subs[0], vi, True, False)
                        for vi in range(3):
                            single_h64(subs[0], vi, True, False)
                            single_h0(subs[1], vi, True, False)
                        for vi in range(3):
                            single_h64(subs[1], vi, True, False)
                        emit_pairs([subs[0]], False, True)
                        emit_pairs([subs[1]], False, True)
                    elif last:
                        # sub-major with pairs last: supertile 0 finishes,
                        # evicts and stores while supertile 1 computes, so
                        # the kernel tail is one sub's evict+store.
                        for s in (0, 1):
                            emit_singles([subs[s]], True, False)
                            emit_pairs([subs[s]], False, True)
                            if last:
                                evict(s)
                                base = 2048 * hblk + 1024 * s
                                if s == 0:
                                    nc.sync.dma_start(
                                        o_flat[b, :, base:base + 1024],
                                        st[:, 1024 * s:1024 * s + 1024])
                                else:
                                    nc.scalar.dma_start(
                                        o_flat[b, :, base:base + 512],
                                        st[:, 1024 * s:1024 * s + 512])
                                    nc.sync.dma_start(
                                        o_flat[b, :, base + 512:base + 1024],
                                        st[:, 1024 * s + 512:1024 * s + 1024])
                    else:
                        # all-64-deep tap matmuls (v8 weight layout):
                        # h0 weight col base: u0->0, u1->384, u2->768
                        # h64: u0->768, u1->0, u2->384
                        nwr = {}
                        WOFF = ((0, 768), (384, 0), (768, 384))
                        XOFF = (((0, 0), (1, 1), (0, 1)), ((1, 0), (0, 0), (1, 1)))
                        for v in range(3):
                            for u in range(3):
                                for s2 in (0, 1):
                                    pe2, po2, xsv2, lm2 = subs[s2]
                                    for half in (0, 1):
                                        bank_i, off = XOFF[half][u]
                                        dst = (pe2, po2)[bank_i]
                                        key = (s2, bank_i)
                                        n = nwr.get(key, 0)
                                        nwr[key] = n + 1
                                        p0, p1 = (0, 64) if half == 0 else (64, 128)
                                        wb = WOFF[u][half] + 128 * v
                                        nc.tensor.matmul(
                                            dst, wt[p0:p1, wb:wb + 128],
                                            xsv2[p0:p1, lm2 + off:lm2 + off + 4, v:v + 128],
                                            start=(n == 0),
                                            stop=(n == 8),
                                        )
                    flip = not flip

                    if not last:
                        evict(0)
                        evict(1)
                        nc.scalar.dma_start(
                            o_flat[b, :, 2048 * hblk:2048 * (hblk + 1)], st[:])
    nc.compile()
    return nc


def _get_nc(with_bias: bool, compute: str = None):
    compute = compute or COMPUTE
    key = ("conv", with_bias, compute)
    if key not in _CACHE:
        _CACHE[key] = _build(with_bias, compute)
    return _CACHE[key]


def _prep_inputs(x, weight, bias, with_bias, compute: str = None):
    compute = compute or COMPUTE
    xs = x.reshape(NCORES, BLOC, C, H, W)
    xr = np.empty((NCORES, BLOC, 128, T * W), np.float32)
    xr[:, :, 0:64] = xs[:, :, :, 0::2, :].reshape(NCORES, BLOC, C, T * W)
    xr[:, :, 64:128] = xs[:, :, :, 1::2, :].reshape(NCORES, BLOC, C, T * W)

    wkc = np.ascontiguousarray(weight.transpose(2, 3, 1, 0))  # [u, v, c, k]
    wpk = np.empty((128, 1152), np.float32)
    for v in range(3):
        wpk[0:64, 128 * v:128 * (v + 1)] = wkc[0, v]        # even pair lower: u0
        wpk[64:128, 128 * v:128 * (v + 1)] = wkc[1, v]      # even pair upper: u1
        wpk[0:64, 384 + 128 * v:384 + 128 * (v + 1)] = wkc[1, v]    # odd pair lower: u1
        wpk[64:128, 384 + 128 * v:384 + 128 * (v + 1)] = wkc[2, v]  # odd pair upper: u2
        wpk[0:64, 768 + 128 * v:768 + 128 * (v + 1)] = wkc[2, v]    # even single: u2
        wpk[64:128, 768 + 128 * v:768 + 128 * (v + 1)] = wkc[0, v]  # odd single: u0

    if compute == "bf16":
        import ml_dtypes
        xr = xr.astype(ml_dtypes.bfloat16)
        wpk = wpk.astype(ml_dtypes.bfloat16)
    in_maps = []
    for core in range(NCORES):
        m = {"xloc": xr[core], "wpk": wpk}
        if with_bias:
            m["btile"] = np.tile(bias, (128, 4))  # bias[j] along free dim
        in_maps.append(m)
    return in_maps


def kernel(x, weight, bias):
    from concourse.bass_utils import run_bass_kernel_spmd

    x = np.ascontiguousarray(np.asarray(x, dtype=np.float32))
    weight = np.asarray(weight, dtype=np.float32)
    bias = np.asarray(bias, dtype=np.float32)
    with_bias = bool(np.any(bias))

    nc = _get_nc(with_bias)
    in_maps = _prep_inputs(x, weight, bias, with_bias)
    res = run_bass_kernel_spmd(nc, in_maps, core_ids=list(range(NCORES)))
    out = np.empty((B, K, HO, WO), np.float32)
    for core in range(NCORES):
        out[core * BLOC:(core + 1) * BLOC] = np.asarray(
            res.results[core]["out"], dtype=np.float32)
    return out
